# revision 21
# baseline (speedup 1.0000x reference)
"""Self-contained Trainium2 Bass kernel for the "Attentive" GNN message-passing
problem:

    x: [8192, 256] f32, attn_vectors: [4, 256] f32
    e_h = l2_normalize(attn_vectors[h] * x, axis=-1)        # [H, N, D]
    Y   = concat_h(e_h)                                     # [N, H*D]
    out = (Y @ Y.T) / H                                     # [N, N]

Strategy (8 NeuronCores, SPMD, no collectives):
  - out is symmetric: only the upper triangle is computed on-device, the host
    mirrors it.  Row blocks are dealt block-cyclically: core c owns global
    128-row blocks {8i + c}; its i-th block computes column panels [2i, 15].
    Every core runs the IDENTICAL program (k_p = p//2 + 1 blocks per panel,
    72 of the 128 baseline blocks per core).
  - Main matmul runs in fp8 e4m3 with DoubleRow perf mode (two 128-deep
    k-chunks per pass, 2x bf16 throughput, measured 216 ns per
    [K=256]x[128x512]); PSUM accumulates f32; out is written f16 and upcast
    on the host (tolerance gate is 2e-2; this pipeline measures ~1.3e-2).
  - x arrives pre-transposed/bf16 from the host (xT [256, 8192]); per-panel
    loads are plain strided DMAs, so there are no PE transposes or casts.
  - Row norms: xsq = Act Square(xT); pn[h, rows] via [4,512]-output bf16
    matmuls (w_sq stationary); rn = sqrt(recip_approx_fast(pn))*S2 =
    S2/||a_h*x_row|| (recip first: the custom DVE op is f32-only, the Act
    sqrt then casts to bf16 for free); rn bounces through DRAM and returns
    as a partition-broadcast DMA (step-0 partition APs are legal on DRAM).
  - Panels are processed in REVERSE order (15..0): the k=8 panels run while
    the rhs lookahead fills, and the rhs-starved k=1 panels land in the
    tail where the PE has nothing else to do anyway.
  - fp8 scales: lhsT = x*a^2*r*(S1/4), rhs = x*r*S2, with S1=64, S2=16 and
    S1/(4*S2) == 1 folded into asq; the PSUM->SBUF copy applies 1/(S1*S2).
  - Elementwise work is spread: rhs panel builds on DVE + GpSimd, squares +
    sqrt + most output copies on Scalar (sqrt/copy/square share one act
    table), recip + some copies on DVE.
"""

from contextlib import ExitStack

import numpy as np

N, D, H = 8192, 256, 4
NCORES = 8
P = 128
PANEL = 512
NPANELS = N // PANEL  # 16
NBLK = 8  # local row blocks per core
CHD = D // P  # 2
KCH = H * CHD  # 8 contraction chunks of 128
NPAIR = KCH // 2  # 4 DoubleRow pairs
S1 = 64.0
S2 = 16.0
OSCALE = 1.0 / (S1 * S2)
PIPE = 6  # panels of prepass lookahead
# panels whose rhs build runs on GpSimd instead of DVE
POOL_RHS = {11, 8}

_COMPILED = {}


def _build_bass():
    import concourse.bass as bass
    import concourse.tile as tile
    from concourse import bacc, mybir

    f32 = mybir.dt.float32
    bf16 = mybir.dt.bfloat16
    f16 = mybir.dt.float16
    fp8 = mybir.dt.float8e4
    DR = mybir.MatmulPerfMode.DoubleRow
    Sqrt = mybir.ActivationFunctionType.Sqrt
    Square = mybir.ActivationFunctionType.Square

    nc = bacc.Bacc(
        "TRN2",
        target_bir_lowering=False,
        debug=False,
        enable_asserts=False,
        num_devices=NCORES,
    )
    xT_t = nc.dram_tensor("xT", [D, N], bf16, kind="ExternalInput")
    xo_t = nc.dram_tensor("xT_own", [D, NBLK * P], bf16, kind="ExternalInput")
    ws_t = nc.dram_tensor("w_sq", [P, CHD, H], bf16, kind="ExternalInput")
    aq_t = nc.dram_tensor("asq", [P, KCH], f32, kind="ExternalInput")
    out_t = nc.dram_tensor("out", [NBLK * P, N], f16, kind="ExternalOutput")

    xTa, out = xT_t.ap(), out_t.ap()

    with tile.TileContext(nc) as tc, ExitStack() as ctx, nc.allow_low_precision(
        "fp8 kernel by design"
    ):
        consts = ctx.enter_context(tc.tile_pool(name="consts", bufs=1))
        xtp = ctx.enter_context(tc.tile_pool(name="xtp", bufs=10))
        sqp = ctx.enter_context(tc.tile_pool(name="sqp", bufs=4))
        smal = ctx.enter_context(tc.tile_pool(name="smal", bufs=3))
        bcp = ctx.enter_context(tc.tile_pool(name="bcp", bufs=8))
        rhsp = ctx.enter_context(tc.tile_pool(name="rhsp", bufs=7))
        outp = ctx.enter_context(tc.tile_pool(name="outp", bufs=2))
        tmpp = ctx.enter_context(tc.tile_pool(name="tmpp", bufs=2))
        dram = ctx.enter_context(tc.tile_pool(name="dram", bufs=1, space="DRAM"))
        ps_n = ctx.enter_context(tc.tile_pool(name="ps_n", bufs=2, space="PSUM"))
        ps_o = ctx.enter_context(tc.tile_pool(name="ps_o", bufs=5, space="PSUM"))

        # lhsT: [128, block r, pair kk, t, m] fp8  (pair stride = 128, the
        # layout the dual-fp8 ldweights path was validated with)
        lhsT = consts.tile([P, NBLK, NPAIR, 2, P], fp8)
        w_sq = consts.tile([P, CHD, H], bf16)
        asq = consts.tile([P, KCH], f32)
        xTown = consts.tile([P, CHD, NBLK, P], bf16)

        xts = {}
        bcs = {}
        rhss = {}
        sqs = {}
        pns = {}

        def load_panel(p):
            t = xtp.tile([P, CHD, PANEL], bf16, tag="xt")
            nc.sync.dma_start(
                t[:],
                xTa[:, p * PANEL : (p + 1) * PANEL].rearrange(
                    "(c q) n -> q c n", q=P
                ),
            )
            xts[p] = t

        def square(p):
            sq = sqp.tile([P, CHD, PANEL], bf16, tag="sq")
            nc.scalar.activation(sq[:], xts[p][:], Square)
            sqs[p] = sq

        def norm_mm(p):
            pn = ps_n.tile([4, PANEL], f32, tag="pn")
            for c in range(CHD):
                nc.tensor.matmul(
                    pn[:],
                    w_sq[:, c, :],
                    sqs[p][:, c, :],
                    start=(c == 0),
                    stop=(c == CHD - 1),
                )
            pns[p] = pn

        def rnorm_bounce(p):
            # rn = S2/sqrt(pn): DVE reciprocal first (needs f32 in/out),
            # then Act sqrt with scale=S2^2 casting to bf16 for free.
            s = smal.tile([4, PANEL], f32, tag="s")
            nc.vector.reciprocal_approx_fast(s[:], pns[p][:])
            rn = smal.tile([4, PANEL], bf16, tag="rn")
            nc.scalar.activation(rn[:], s[:], Sqrt, scale=S2 * S2)
            rnd = dram.tile([4, PANEL], bf16, name=f"rnd{p}")
            nc.sync.dma_start(rnd[:], rn[:])
            bc = bcp.tile([P, H, PANEL], bf16, tag="bc")
            src = bass.AP(rnd.tensor, rnd.offset, [[0, P], [PANEL, H], [1, PANEL]])
            nc.sync.dma_start(bc[:], src)
            bcs[p] = bc

        def rhs_build(p):
            # rhs[q, h, c, n] = xT[q, c, n] * bc[q, h, n], emitted as two
            # half-builds (heads 0-1 / 2-3) so the PE can start a panel's
            # first k-pairs before the whole panel is scaled.
            rhs = rhsp.tile([P, NPAIR, 2, PANEL], fp8, tag="rhs")
            xt = xts[p]
            bc = bcs[p]
            eng = nc.gpsimd if p in POOL_RHS else nc.vector
            for hh in range(2):
                in0 = bass.AP(
                    xt.tensor,
                    xt.offset,
                    [list(xt.ap[0]), [0, 2], [PANEL, CHD], [1, PANEL]],
                )
                in1 = bass.AP(
                    bc.tensor,
                    bc.offset + 2 * hh * PANEL,
                    [list(bc.ap[0]), [PANEL, 2], [0, CHD], [1, PANEL]],
                )
                eng.tensor_tensor(
                    rhs[:, 2 * hh : 2 * hh + 2, :, :], in0, in1,
                    mybir.AluOpType.mult,
                )
            rhss[p] = rhs

        # ---- prologue (panels processed in REVERSE order 15..0) -----------
        # DMA trigger order: the panel-15/14 loads and xTown lead (they gate
        # the startup chains), small consts after.
        ORDER = list(range(NPANELS - 1, -1, -1))
        load_panel(ORDER[0])
        load_panel(ORDER[1])
        nc.sync.dma_start(
            xTown[:],
            xo_t.ap()[:].rearrange("(c q) (i n) -> q c i n", q=P, n=P),
        )
        nc.sync.dma_start(w_sq[:], ws_t.ap()[:])
        nc.sync.dma_start(asq[:], aq_t.ap()[:])
        for j in range(2, PIPE + 1):
            load_panel(ORDER[j])

        # Dummy tiny sqrt: pins the act table that holds sqrt+square+copy
        # before the panel-0 chain, so no reload lands on the critical path.
        dummy = smal.tile([4, 8], f32, tag="dummy")
        nc.scalar.activation(dummy[:], asq[:4, :], Sqrt)

        # Panel 15 prepass chain first: rhs(15) is the main-loop gate.
        for j in range(1):
            square(ORDER[j])
            norm_mm(ORDER[j])
            rnorm_bounce(ORDER[j])

        # Own-row norm pass (gates only the lhsT build).
        xsq_own = consts.tile([P, CHD, NBLK, P], bf16)
        nc.scalar.activation(xsq_own[:], xTown[:], Square)
        rnd_own = dram.tile([4, NBLK * P], bf16, name="rnd_own")
        for half in range(2):
            pn = ps_n.tile([4, PANEL], f32, tag="pn")
            for c in range(CHD):
                nc.tensor.matmul(
                    pn[:],
                    w_sq[:, c, :],
                    xsq_own[:, c, 4 * half : 4 * half + 4, :],
                    start=(c == 0),
                    stop=(c == CHD - 1),
                )
            s = smal.tile([4, PANEL], f32, tag="s")
            nc.vector.reciprocal_approx_fast(s[:], pn[:])
            rn = smal.tile([4, PANEL], bf16, tag="rn")
            nc.scalar.activation(rn[:], s[:], Sqrt, scale=S2 * S2)
            nc.sync.dma_start(
                rnd_own[:, half * PANEL : (half + 1) * PANEL], rn[:]
            )
        bc_own = consts.tile([P, H, NBLK * P], bf16)
        nc.sync.dma_start(
            bc_own[:],
            bass.AP(
                rnd_own.tensor,
                rnd_own.offset,
                [[0, P], [NBLK * P, H], [1, NBLK * P]],
            ),
        )

        pass

        # tmp_all[kc] = xTown * asq[kc] for all 8 blocks: depends only on
        # inputs, so it runs in the prologue off every critical path.
        tmp_all = consts.tile([P, KCH, NBLK, P], bf16)
        for kc in range(KCH):
            nc.vector.tensor_scalar_mul(
                tmp_all[:, kc, :, :],
                xTown[:, kc % CHD, :, :],
                asq[:, kc : kc + 1],
            )

        rhs_build(ORDER[0])

        # lhsT build in block PAIRS (r, r+1): block r is first consumed at
        # panel 2r, so pair b covers panels 4b..; built just in time.
        def lhsT_build(pair, h, c, eng):
            blks = slice(2 * pair, 2 * pair + 2)
            kc = h * CHD + c
            eng.tensor_tensor(
                lhsT[:, blks, h, c, :],
                tmp_all[:, kc, blks, :],
                bc_own[:, h, 2 * pair * P : (2 * pair + 2) * P].rearrange(
                    "q (i n) -> q i n", n=P
                ),
                mybir.AluOpType.mult,
            )

        # pair 0 on GpSimd (idle at startup); pairs 1-3 on DVE right after
        # rhs(15): main(15) consumes blocks in order r=0..7, so later pairs
        # may lag slightly without stalling the PE for long.
        for h in range(H):
            for c in range(CHD):
                lhsT_build(0, h, c, nc.gpsimd)
        for pair in (3,):
            for h in range(H):
                for c in range(CHD):
                    lhsT_build(pair, h, c, nc.gpsimd)
        for pair in (1, 2):
            for h in range(H):
                for c in range(CHD):
                    lhsT_build(pair, h, c, nc.vector)
        # rest of the prologue prepass pipeline, off the critical path
        for j in range(1, PIPE):
            square(ORDER[j])
            norm_mm(ORDER[j])
            rnorm_bounce(ORDER[j])
        square(ORDER[PIPE])

        # ---- main loop over 16 column panels, heaviest (p=15) first --------
        copy_cnt = 0
        for j in range(NPANELS):
            p = ORDER[j]
            if j + PIPE + 1 < NPANELS:
                load_panel(ORDER[j + PIPE + 1])
            if j + 1 < NPANELS and ORDER[j + 1] not in POOL_RHS:
                rhs_build(ORDER[j + 1])
            if j + 2 < NPANELS and ORDER[j + 2] in POOL_RHS:
                rhs_build(ORDER[j + 2])
            k_p = p // 2 + 1
            rhs = rhss.pop(p)
            ot = outp.tile([P, NBLK, PANEL], f16, tag="ot")
            for r in range(k_p):
                acc = ps_o.tile([P, PANEL], f32, tag="acc")
                for kk in range(NPAIR):
                    nc.tensor.matmul(
                        acc[:],
                        lhsT[:, r, kk, :, :],
                        rhs[:, kk, :, :],
                        start=(kk == 0),
                        stop=(kk == NPAIR - 1),
                        perf_mode=DR,
                    )
                on_dve = (copy_cnt % 2 == 0) if p <= 1 else False
                if on_dve:
                    nc.vector.tensor_scalar_mul(ot[:, r, :], acc[:], OSCALE)
                else:
                    nc.scalar.mul(ot[:, r, :], acc[:], OSCALE)
                copy_cnt += 1
                if p <= 1:
                    nc.sync.dma_start(
                        out[r * P : (r + 1) * P, p * PANEL : (p + 1) * PANEL],
                        ot[:, r, :],
                    )
            if p > 1:
                nc.sync.dma_start(
                    out[0 : k_p * P, p * PANEL : (p + 1) * PANEL].rearrange(
                        "(r q) n -> q r n", q=P
                    ),
                    ot[:, :k_p, :],
                )

            # rest of the prepass pipeline (norm after this panel's mains:
            # PIPE=6 gives plenty of slack and the PE queue never blocks)
            if j + PIPE < NPANELS:
                norm_mm(ORDER[j + PIPE])
                rnorm_bounce(ORDER[j + PIPE])
            if j + PIPE + 1 < NPANELS:
                square(ORDER[j + PIPE + 1])

    nc.compile()
    return nc


def _get_compiled():
    if "nc" not in _COMPILED:
        _COMPILED["nc"] = _build_bass()
    return _COMPILED["nc"]


def host_side_inputs(x, attn):
    """Per-core input maps. Host work is layout/dtype marshaling of x plus
    tiny functions of attn_vectors; all real compute (norms, scaling, the
    N^2 matmul) runs on device."""
    import ml_dtypes

    bf16 = ml_dtypes.bfloat16
    xT = np.ascontiguousarray(x.T).astype(bf16)  # [256, 8192]
    w_sq = np.zeros((P, CHD, H), dtype=np.float32)
    asq = np.zeros((P, KCH), dtype=np.float32)
    for c in range(CHD):
        for h in range(H):
            w_sq[:, c, h] = attn[h, c * P : (c + 1) * P] ** 2
            asq[:, h * CHD + c] = attn[h, c * P : (c + 1) * P] ** 2
    w_sq = w_sq.astype(bf16)
    maps = []
    for c in range(NCORES):
        cols = np.concatenate(
            [
                np.arange((8 * i + c) * P, (8 * i + c + 1) * P)
                for i in range(NBLK)
            ]
        )
        maps.append(
            {
                "xT": xT,
                "xT_own": np.ascontiguousarray(xT[:, cols]),
                "w_sq": w_sq,
                "asq": asq,
            }
        )
    return maps


def assemble_output(results):
    """Scatter per-core row blocks into the full matrix and mirror the
    computed upper triangle."""
    full = np.empty((N, N), dtype=np.float16)
    for c in range(NCORES):
        o = results[c]["out"]
        for i in range(NBLK):
            full[(8 * i + c) * P : (8 * i + c + 1) * P, :] = o[
                i * P : (i + 1) * P, :
            ]
    f = full.astype(np.float32)
    return np.triu(f) + np.triu(f, 1).T


def kernel(**inputs) -> np.ndarray:
    from concourse import bass_utils

    x = np.ascontiguousarray(np.asarray(inputs["x"], dtype=np.float32))
    attn = np.ascontiguousarray(
        np.asarray(inputs["attn_vectors"], dtype=np.float32)
    )
    nc = _get_compiled()
    res = bass_utils.run_bass_kernel_spmd(
        nc, host_side_inputs(x, attn), core_ids=list(range(NCORES))
    )
    return assemble_output(res.results)


# revision 22
# speedup vs baseline: 1.1820x; 1.1820x over previous
"""Self-contained Trainium2 Bass kernel for the "Attentive" GNN message-passing
problem:

    x: [8192, 256] f32, attn_vectors: [4, 256] f32
    e_h = l2_normalize(attn_vectors[h] * x, axis=-1)        # [H, N, D]
    Y   = concat_h(e_h)                                     # [N, H*D]
    out = (Y @ Y.T) / H                                     # [N, N]

Strategy (8 NeuronCores, SPMD, no collectives):
  - out is symmetric: only the upper triangle is computed on-device, the host
    mirrors it.  Row blocks are dealt block-cyclically: core c owns global
    128-row blocks {8i + c}; its i-th block computes column panels [2i, 15].
    Every core runs the IDENTICAL program (k_p = p//2 + 1 blocks per panel,
    72 of the 128 baseline blocks per core).
  - Main matmul runs in fp8 e4m3 with DoubleRow perf mode (two 128-deep
    k-chunks per pass, 2x bf16 throughput, measured 216 ns per
    [K=256]x[128x512]); PSUM accumulates f32; out is written f16 and upcast
    on the host (tolerance gate is 2e-2; this pipeline measures ~1.3e-2).
  - x arrives pre-transposed/bf16 from the host (xT [256, 8192]); per-panel
    loads are plain strided DMAs, so there are no PE transposes or casts.
  - Row norms: xsq = Act Square(xT); pn[h, rows] via [4,512]-output bf16
    matmuls (w_sq stationary); rn = sqrt(recip_approx_fast(pn))*S2 =
    S2/||a_h*x_row|| (recip first: the custom DVE op is f32-only, the Act
    sqrt then casts to bf16 for free); rn bounces through DRAM and returns
    as a partition-broadcast DMA (step-0 partition APs are legal on DRAM).
  - Panels are processed in REVERSE order (15..0): the k=8 panels run while
    the rhs lookahead fills, and the rhs-starved k=1 panels land in the
    tail where the PE has nothing else to do anyway.
  - fp8 scales: lhsT = x*a^2*r*(S1/4), rhs = x*r*S2, with S1=64, S2=16 and
    S1/(4*S2) == 1 folded into asq; the PSUM->SBUF copy applies 1/(S1*S2).
  - Elementwise work is spread: rhs panel builds on DVE + GpSimd, squares +
    sqrt + most output copies on Scalar (sqrt/copy/square share one act
    table), recip + some copies on DVE.
"""

from contextlib import ExitStack

import numpy as np

N, D, H = 8192, 256, 4
NCORES = 8
P = 128
PANEL = 512
NPANELS = N // PANEL  # 16
NBLK = 8  # local row blocks per core
CHD = D // P  # 2
KCH = H * CHD  # 8 contraction chunks of 128
NPAIR = KCH // 2  # 4 DoubleRow pairs
S1 = 64.0
S2 = 16.0
OSCALE = 1.0 / (S1 * S2)
PIPE = 6  # panels of prepass lookahead
# panels whose rhs build runs on GpSimd instead of DVE
POOL_RHS = {12, 9}

_COMPILED = {}


def _build_bass():
    import concourse.bass as bass
    import concourse.tile as tile
    from concourse import bacc, mybir

    f32 = mybir.dt.float32
    bf16 = mybir.dt.bfloat16
    f16 = mybir.dt.float16
    fp8 = mybir.dt.float8e4
    DR = mybir.MatmulPerfMode.DoubleRow
    Sqrt = mybir.ActivationFunctionType.Sqrt
    Square = mybir.ActivationFunctionType.Square

    nc = bacc.Bacc(
        "TRN2",
        target_bir_lowering=False,
        debug=False,
        enable_asserts=False,
        num_devices=NCORES,
    )
    xT_t = nc.dram_tensor("xT", [D, N], bf16, kind="ExternalInput")
    xo_t = nc.dram_tensor("xT_own", [D, NBLK * P], bf16, kind="ExternalInput")
    ws_t = nc.dram_tensor("w_sq", [P, CHD, H], bf16, kind="ExternalInput")
    aq_t = nc.dram_tensor("asq", [P, KCH], f32, kind="ExternalInput")
    out_t = nc.dram_tensor("out", [NBLK * P, N], f16, kind="ExternalOutput")

    xTa, out = xT_t.ap(), out_t.ap()

    with tile.TileContext(nc) as tc, ExitStack() as ctx, nc.allow_low_precision(
        "fp8 kernel by design"
    ):
        consts = ctx.enter_context(tc.tile_pool(name="consts", bufs=1))
        xtp = ctx.enter_context(tc.tile_pool(name="xtp", bufs=10))
        sqp = ctx.enter_context(tc.tile_pool(name="sqp", bufs=4))
        smal = ctx.enter_context(tc.tile_pool(name="smal", bufs=3))
        bcp = ctx.enter_context(tc.tile_pool(name="bcp", bufs=8))
        rhsp = ctx.enter_context(tc.tile_pool(name="rhsp", bufs=7))
        outp = ctx.enter_context(tc.tile_pool(name="outp", bufs=2))
        tmpp = ctx.enter_context(tc.tile_pool(name="tmpp", bufs=2))
        dram = ctx.enter_context(tc.tile_pool(name="dram", bufs=1, space="DRAM"))
        ps_n = ctx.enter_context(tc.tile_pool(name="ps_n", bufs=2, space="PSUM"))
        ps_o = ctx.enter_context(tc.tile_pool(name="ps_o", bufs=5, space="PSUM"))

        # lhsT: [128, block r, pair kk, t, m] fp8  (pair stride = 128, the
        # layout the dual-fp8 ldweights path was validated with)
        lhsT = consts.tile([P, NBLK, NPAIR, 2, P], fp8)
        w_sq = consts.tile([P, CHD, H], bf16)
        asq = consts.tile([P, KCH], f32)
        xTown = consts.tile([P, CHD, NBLK, P], bf16)

        xts = {}
        bcs = {}
        rhss = {}
        sqs = {}
        pns = {}

        def load_panel(p):
            t = xtp.tile([P, CHD, PANEL], bf16, tag="xt")
            nc.sync.dma_start(
                t[:],
                xTa[:, p * PANEL : (p + 1) * PANEL].rearrange(
                    "(c q) n -> q c n", q=P
                ),
            )
            xts[p] = t

        def square(p):
            sq = sqp.tile([P, CHD, PANEL], bf16, tag="sq")
            nc.scalar.activation(sq[:], xts[p][:], Square)
            sqs[p] = sq

        def norm_mm(p):
            pn = ps_n.tile([4, PANEL], f32, tag="pn")
            for c in range(CHD):
                nc.tensor.matmul(
                    pn[:],
                    w_sq[:, c, :],
                    sqs[p][:, c, :],
                    start=(c == 0),
                    stop=(c == CHD - 1),
                )
            pns[p] = pn

        def rnorm_bounce(p):
            # rn = S2/sqrt(pn): DVE reciprocal first (needs f32 in/out),
            # then Act sqrt with scale=S2^2 casting to bf16 for free.
            s = smal.tile([4, PANEL], f32, tag="s")
            nc.vector.reciprocal_approx_fast(s[:], pns[p][:])
            rn = smal.tile([4, PANEL], bf16, tag="rn")
            nc.scalar.activation(rn[:], s[:], Sqrt, scale=S2 * S2)
            rnd = dram.tile([4, PANEL], bf16, name=f"rnd{p}")
            nc.sync.dma_start(rnd[:], rn[:])
            bc = bcp.tile([P, H, PANEL], bf16, tag="bc")
            src = bass.AP(rnd.tensor, rnd.offset, [[0, P], [PANEL, H], [1, PANEL]])
            nc.sync.dma_start(bc[:], src)
            bcs[p] = bc

        def rhs_build(p):
            # rhs[q, h, c, n] = xT[q, c, n] * bc[q, h, n]
            rhs = rhsp.tile([P, NPAIR, 2, PANEL], fp8, tag="rhs")
            xt = xts[p]
            bc = bcs[p]
            in0 = bass.AP(
                xt.tensor,
                xt.offset,
                [list(xt.ap[0]), [0, H], [PANEL, CHD], [1, PANEL]],
            )
            in1 = bass.AP(
                bc.tensor,
                bc.offset,
                [list(bc.ap[0]), [PANEL, H], [0, CHD], [1, PANEL]],
            )
            eng = nc.gpsimd if p in POOL_RHS else nc.vector
            eng.tensor_tensor(rhs[:], in0, in1, mybir.AluOpType.mult)
            rhss[p] = rhs

        # ---- prologue (panels processed in REVERSE order 15..0) -----------
        # DMA trigger order: the panel-15/14 loads and xTown lead (they gate
        # the startup chains), small consts after.
        ORDER = list(range(NPANELS - 1, -1, -1))
        load_panel(ORDER[0])
        load_panel(ORDER[1])
        nc.sync.dma_start(
            xTown[:],
            xo_t.ap()[:].rearrange("(c q) (i n) -> q c i n", q=P, n=P),
        )
        nc.sync.dma_start(w_sq[:], ws_t.ap()[:])
        nc.sync.dma_start(asq[:], aq_t.ap()[:])
        for j in range(2, PIPE + 1):
            load_panel(ORDER[j])

        # Dummy tiny sqrt: pins the act table that holds sqrt+square+copy
        # before the panel-0 chain, so no reload lands on the critical path.
        dummy = smal.tile([4, 8], f32, tag="dummy")
        nc.scalar.activation(dummy[:], asq[:4, :], Sqrt)

        # Panel 15 prepass chain first: rhs(15) is the main-loop gate.
        square(ORDER[0])
        norm_mm(ORDER[0])
        rnorm_bounce(ORDER[0])

        # Own-row norm pass (gates only the lhsT build).
        xsq_own = consts.tile([P, CHD, NBLK, P], bf16)
        nc.scalar.activation(xsq_own[:], xTown[:], Square)
        rnd_own = dram.tile([4, NBLK * P], bf16, name="rnd_own")
        for half in range(2):
            pn = ps_n.tile([4, PANEL], f32, tag="pn")
            for c in range(CHD):
                nc.tensor.matmul(
                    pn[:],
                    w_sq[:, c, :],
                    xsq_own[:, c, 4 * half : 4 * half + 4, :],
                    start=(c == 0),
                    stop=(c == CHD - 1),
                )
            s = smal.tile([4, PANEL], f32, tag="s")
            nc.vector.reciprocal_approx_fast(s[:], pn[:])
            rn = smal.tile([4, PANEL], bf16, tag="rn")
            nc.scalar.activation(rn[:], s[:], Sqrt, scale=S2 * S2)
            nc.sync.dma_start(
                rnd_own[:, half * PANEL : (half + 1) * PANEL], rn[:]
            )
        bc_own = consts.tile([P, H, NBLK * P], bf16)
        nc.sync.dma_start(
            bc_own[:],
            bass.AP(
                rnd_own.tensor,
                rnd_own.offset,
                [[0, P], [NBLK * P, H], [1, NBLK * P]],
            ),
        )

        square(ORDER[1])
        norm_mm(ORDER[1])
        rnorm_bounce(ORDER[1])

        # tmp_all[kc] = xTown * asq[kc] for all 8 blocks: depends only on
        # inputs, so it runs in the prologue off every critical path.
        tmp_all = consts.tile([P, KCH, NBLK, P], bf16)
        for kc in range(KCH):
            nc.vector.tensor_scalar_mul(
                tmp_all[:, kc, :, :],
                xTown[:, kc % CHD, :, :],
                asq[:, kc : kc + 1],
            )

        rhs_build(ORDER[0])

        # lhsT build in block PAIRS (r, r+1): block r is first consumed at
        # panel 2r, so pair b covers panels 4b..; built just in time.
        def lhsT_build(pair, h, c, eng):
            blks = slice(2 * pair, 2 * pair + 2)
            kc = h * CHD + c
            eng.tensor_tensor(
                lhsT[:, blks, h, c, :],
                tmp_all[:, kc, blks, :],
                bc_own[:, h, 2 * pair * P : (2 * pair + 2) * P].rearrange(
                    "q (i n) -> q i n", n=P
                ),
                mybir.AluOpType.mult,
            )

        # pair 0 on GpSimd (idle at startup); pairs 1-3 on DVE right after
        # rhs(15): main(15) consumes blocks in order r=0..7, so later pairs
        # may lag slightly without stalling the PE for long.
        for h in range(H):
            for c in range(CHD):
                lhsT_build(0, h, c, nc.gpsimd)
        for pair in (3,):
            for h in range(H):
                for c in range(CHD):
                    lhsT_build(pair, h, c, nc.gpsimd)
        for pair in (1, 2):
            for h in range(H):
                for c in range(CHD):
                    lhsT_build(pair, h, c, nc.vector)
        # remaining prologue prepasses, off the rhs(15) critical path
        for j in range(2, PIPE):
            square(ORDER[j])
            norm_mm(ORDER[j])
            rnorm_bounce(ORDER[j])
        square(ORDER[PIPE])

        # ---- main loop over 16 column panels, heaviest (p=15) first --------
        copy_cnt = 0
        for j in range(NPANELS):
            p = ORDER[j]
            if j + PIPE + 1 < NPANELS:
                load_panel(ORDER[j + PIPE + 1])
            if j + 1 < NPANELS and ORDER[j + 1] not in POOL_RHS:
                rhs_build(ORDER[j + 1])
            if j + 2 < NPANELS and ORDER[j + 2] in POOL_RHS:
                rhs_build(ORDER[j + 2])
            # norm matmul BEFORE this panel's main matmuls: its square was
            # issued an iteration earlier, so the PE never waits here and
            # the dependent Act sqrt clears before this panel's copies.
            if j + PIPE < NPANELS:
                norm_mm(ORDER[j + PIPE])

            k_p = p // 2 + 1
            rhs = rhss.pop(p)
            ot = outp.tile([P, NBLK, PANEL], f16, tag="ot")
            for r in range(k_p):
                acc = ps_o.tile([P, PANEL], f32, tag="acc")
                for kk in range(NPAIR):
                    nc.tensor.matmul(
                        acc[:],
                        lhsT[:, r, kk, :, :],
                        rhs[:, kk, :, :],
                        start=(kk == 0),
                        stop=(kk == NPAIR - 1),
                        perf_mode=DR,
                    )
                on_dve = (copy_cnt % 2 == 0) if p <= 1 else False
                if on_dve:
                    nc.vector.tensor_scalar_mul(ot[:, r, :], acc[:], OSCALE)
                else:
                    nc.scalar.mul(ot[:, r, :], acc[:], OSCALE)
                copy_cnt += 1
                if p <= 1:
                    nc.sync.dma_start(
                        out[r * P : (r + 1) * P, p * PANEL : (p + 1) * PANEL],
                        ot[:, r, :],
                    )
            if p > 1:
                nc.sync.dma_start(
                    out[0 : k_p * P, p * PANEL : (p + 1) * PANEL].rearrange(
                        "(r q) n -> q r n", q=P
                    ),
                    ot[:, :k_p, :],
                )

            # rest of the prepass pipeline
            if j + PIPE < NPANELS:
                rnorm_bounce(ORDER[j + PIPE])
            if j + PIPE + 1 < NPANELS:
                square(ORDER[j + PIPE + 1])

    nc.compile()
    return nc


def _get_compiled():
    if "nc" not in _COMPILED:
        _COMPILED["nc"] = _build_bass()
    return _COMPILED["nc"]


def host_side_inputs(x, attn):
    """Per-core input maps. Host work is layout/dtype marshaling of x plus
    tiny functions of attn_vectors; all real compute (norms, scaling, the
    N^2 matmul) runs on device."""
    import ml_dtypes

    bf16 = ml_dtypes.bfloat16
    xT = np.ascontiguousarray(x.T).astype(bf16)  # [256, 8192]
    w_sq = np.zeros((P, CHD, H), dtype=np.float32)
    asq = np.zeros((P, KCH), dtype=np.float32)
    for c in range(CHD):
        for h in range(H):
            w_sq[:, c, h] = attn[h, c * P : (c + 1) * P] ** 2
            asq[:, h * CHD + c] = attn[h, c * P : (c + 1) * P] ** 2
    w_sq = w_sq.astype(bf16)
    maps = []
    for c in range(NCORES):
        cols = np.concatenate(
            [
                np.arange((8 * i + c) * P, (8 * i + c + 1) * P)
                for i in range(NBLK)
            ]
        )
        maps.append(
            {
                "xT": xT,
                "xT_own": np.ascontiguousarray(xT[:, cols]),
                "w_sq": w_sq,
                "asq": asq,
            }
        )
    return maps


def assemble_output(results):
    """Scatter per-core row blocks into the full matrix and mirror the
    computed upper triangle."""
    full = np.empty((N, N), dtype=np.float16)
    for c in range(NCORES):
        o = results[c]["out"]
        for i in range(NBLK):
            full[(8 * i + c) * P : (8 * i + c + 1) * P, :] = o[
                i * P : (i + 1) * P, :
            ]
    f = full.astype(np.float32)
    return np.triu(f) + np.triu(f, 1).T


def kernel(**inputs) -> np.ndarray:
    from concourse import bass_utils

    x = np.ascontiguousarray(np.asarray(inputs["x"], dtype=np.float32))
    attn = np.ascontiguousarray(
        np.asarray(inputs["attn_vectors"], dtype=np.float32)
    )
    nc = _get_compiled()
    res = bass_utils.run_bass_kernel_spmd(
        nc, host_side_inputs(x, attn), core_ids=list(range(NCORES))
    )
    return assemble_output(res.results)


# revision 23
# speedup vs baseline: 1.2284x; 1.0392x over previous
"""Self-contained Trainium2 Bass kernel for the "Attentive" GNN message-passing
problem:

    x: [8192, 256] f32, attn_vectors: [4, 256] f32
    e_h = l2_normalize(attn_vectors[h] * x, axis=-1)        # [H, N, D]
    Y   = concat_h(e_h)                                     # [N, H*D]
    out = (Y @ Y.T) / H                                     # [N, N]

Strategy (8 NeuronCores, SPMD, no collectives):
  - out is symmetric: only the upper triangle is computed on-device, the host
    mirrors it.  Row blocks are dealt block-cyclically: core c owns global
    128-row blocks {8i + c}; its i-th block computes column panels [2i, 15].
    Every core runs the IDENTICAL program (k_p = p//2 + 1 blocks per panel,
    72 of the 128 baseline blocks per core).
  - Main matmul runs in fp8 e4m3 with DoubleRow perf mode (two 128-deep
    k-chunks per pass, 2x bf16 throughput, measured 216 ns per
    [K=256]x[128x512]); PSUM accumulates f32; out is written f16 and upcast
    on the host (tolerance gate is 2e-2; this pipeline measures ~1.3e-2).
  - x arrives pre-transposed/bf16 from the host (xT [256, 8192]); per-panel
    loads are plain strided DMAs, so there are no PE transposes or casts.
  - Row norms: xsq = Act Square(xT); pn[h, rows] via [4,512]-output bf16
    matmuls (w_sq stationary); rn = sqrt(recip_approx_fast(pn))*S2 =
    S2/||a_h*x_row|| (recip first: the custom DVE op is f32-only, the Act
    sqrt then casts to bf16 for free); rn bounces through DRAM and returns
    as a partition-broadcast DMA (step-0 partition APs are legal on DRAM).
  - Panels are processed in REVERSE order (15..0): the k=8 panels run while
    the rhs lookahead fills, and the rhs-starved k=1 panels land in the
    tail where the PE has nothing else to do anyway.
  - fp8 scales: lhsT = x*a^2*r*(S1/4), rhs = x*r*S2, with S1=64, S2=16 and
    S1/(4*S2) == 1 folded into asq; the PSUM->SBUF copy applies 1/(S1*S2).
  - Elementwise work is spread: rhs panel builds on DVE + GpSimd, squares +
    sqrt + most output copies on Scalar (sqrt/copy/square share one act
    table), recip + some copies on DVE.
"""

from contextlib import ExitStack

import numpy as np

N, D, H = 8192, 256, 4
NCORES = 8
P = 128
PANEL = 512
NPANELS = N // PANEL  # 16
NBLK = 8  # local row blocks per core
CHD = D // P  # 2
KCH = H * CHD  # 8 contraction chunks of 128
NPAIR = KCH // 2  # 4 DoubleRow pairs
S1 = 64.0
S2 = 16.0
OSCALE = 1.0 / (S1 * S2)
PIPE = 6  # panels of prepass lookahead
# panels whose rhs build runs on GpSimd instead of DVE
POOL_RHS = {13, 10, 7}

_COMPILED = {}


def _build_bass():
    import concourse.bass as bass
    import concourse.tile as tile
    from concourse import bacc, mybir

    f32 = mybir.dt.float32
    bf16 = mybir.dt.bfloat16
    f16 = mybir.dt.float16
    fp8 = mybir.dt.float8e4
    DR = mybir.MatmulPerfMode.DoubleRow
    Sqrt = mybir.ActivationFunctionType.Sqrt
    Square = mybir.ActivationFunctionType.Square

    nc = bacc.Bacc(
        "TRN2",
        target_bir_lowering=False,
        debug=False,
        enable_asserts=False,
        num_devices=NCORES,
    )
    xT_t = nc.dram_tensor("xT", [D, N], bf16, kind="ExternalInput")
    xo_t = nc.dram_tensor("xT_own", [D, NBLK * P], bf16, kind="ExternalInput")
    ws_t = nc.dram_tensor("w_sq", [P, CHD, H], bf16, kind="ExternalInput")
    aq_t = nc.dram_tensor("asq", [P, KCH], f32, kind="ExternalInput")
    out_t = nc.dram_tensor("out", [NBLK * P, N], f16, kind="ExternalOutput")

    xTa, out = xT_t.ap(), out_t.ap()

    with tile.TileContext(nc) as tc, ExitStack() as ctx, nc.allow_low_precision(
        "fp8 kernel by design"
    ):
        consts = ctx.enter_context(tc.tile_pool(name="consts", bufs=1))
        xtp = ctx.enter_context(tc.tile_pool(name="xtp", bufs=10))
        sqp = ctx.enter_context(tc.tile_pool(name="sqp", bufs=4))
        smal = ctx.enter_context(tc.tile_pool(name="smal", bufs=3))
        bcp = ctx.enter_context(tc.tile_pool(name="bcp", bufs=8))
        rhsp = ctx.enter_context(tc.tile_pool(name="rhsp", bufs=7))
        outp = ctx.enter_context(tc.tile_pool(name="outp", bufs=2))
        tmpp = ctx.enter_context(tc.tile_pool(name="tmpp", bufs=2))
        dram = ctx.enter_context(tc.tile_pool(name="dram", bufs=1, space="DRAM"))
        ps_n = ctx.enter_context(tc.tile_pool(name="ps_n", bufs=2, space="PSUM"))
        ps_o = ctx.enter_context(tc.tile_pool(name="ps_o", bufs=5, space="PSUM"))

        # lhsT: [128, block r, pair kk, t, m] fp8  (pair stride = 128, the
        # layout the dual-fp8 ldweights path was validated with)
        lhsT = consts.tile([P, NBLK, NPAIR, 2, P], fp8)
        w_sq = consts.tile([P, CHD, H], bf16)
        asq = consts.tile([P, KCH], f32)
        xTown = consts.tile([P, CHD, NBLK, P], bf16)

        xts = {}
        bcs = {}
        rhss = {}
        sqs = {}
        pns = {}

        def load_panel(p):
            t = xtp.tile([P, CHD, PANEL], bf16, tag="xt")
            nc.sync.dma_start(
                t[:],
                xTa[:, p * PANEL : (p + 1) * PANEL].rearrange(
                    "(c q) n -> q c n", q=P
                ),
            )
            xts[p] = t

        def square(p):
            sq = sqp.tile([P, CHD, PANEL], bf16, tag="sq")
            nc.scalar.activation(sq[:], xts[p][:], Square)
            sqs[p] = sq

        def norm_mm(p):
            pn = ps_n.tile([4, PANEL], f32, tag="pn")
            for c in range(CHD):
                nc.tensor.matmul(
                    pn[:],
                    w_sq[:, c, :],
                    sqs[p][:, c, :],
                    start=(c == 0),
                    stop=(c == CHD - 1),
                )
            pns[p] = pn

        def rnorm_bounce(p):
            # rn = S2/sqrt(pn): DVE reciprocal first (needs f32 in/out),
            # then Act sqrt with scale=S2^2 casting to bf16 for free.
            s = smal.tile([4, PANEL], f32, tag="s")
            nc.vector.reciprocal_approx_fast(s[:], pns[p][:])
            rn = smal.tile([4, PANEL], bf16, tag="rn")
            nc.scalar.activation(rn[:], s[:], Sqrt, scale=S2 * S2)
            rnd = dram.tile([4, PANEL], bf16, name=f"rnd{p}")
            nc.sync.dma_start(rnd[:], rn[:])
            bc = bcp.tile([P, H, PANEL], bf16, tag="bc")
            src = bass.AP(rnd.tensor, rnd.offset, [[0, P], [PANEL, H], [1, PANEL]])
            nc.sync.dma_start(bc[:], src)
            bcs[p] = bc

        def rhs_build(p):
            # rhs[q, h, c, n] = xT[q, c, n] * bc[q, h, n]
            rhs = rhsp.tile([P, NPAIR, 2, PANEL], fp8, tag="rhs")
            xt = xts[p]
            bc = bcs[p]
            in0 = bass.AP(
                xt.tensor,
                xt.offset,
                [list(xt.ap[0]), [0, H], [PANEL, CHD], [1, PANEL]],
            )
            in1 = bass.AP(
                bc.tensor,
                bc.offset,
                [list(bc.ap[0]), [PANEL, H], [0, CHD], [1, PANEL]],
            )
            eng = nc.gpsimd if p in POOL_RHS else nc.vector
            eng.tensor_tensor(rhs[:], in0, in1, mybir.AluOpType.mult)
            rhss[p] = rhs

        # ---- prologue (panels processed in REVERSE order 15..0) -----------
        # DMA trigger order: the panel-15/14 loads and xTown lead (they gate
        # the startup chains), small consts after.
        ORDER = list(range(NPANELS - 1, -1, -1))
        load_panel(ORDER[0])
        load_panel(ORDER[1])
        nc.sync.dma_start(
            xTown[:],
            xo_t.ap()[:].rearrange("(c q) (i n) -> q c i n", q=P, n=P),
        )
        nc.sync.dma_start(w_sq[:], ws_t.ap()[:])
        nc.sync.dma_start(asq[:], aq_t.ap()[:])
        for j in range(2, PIPE + 1):
            load_panel(ORDER[j])

        # Dummy tiny sqrt: pins the act table that holds sqrt+square+copy
        # before the panel-0 chain, so no reload lands on the critical path.
        dummy = smal.tile([4, 8], f32, tag="dummy")
        nc.scalar.activation(dummy[:], asq[:4, :], Sqrt)

        # Panel 15/14 prepass chains first: rhs(15) is the main-loop gate.
        for j in range(2):
            square(ORDER[j])
            norm_mm(ORDER[j])
            rnorm_bounce(ORDER[j])

        # Own-row norm pass (gates only the lhsT build).
        xsq_own = consts.tile([P, CHD, NBLK, P], bf16)
        nc.scalar.activation(xsq_own[:], xTown[:], Square)
        rnd_own = dram.tile([4, NBLK * P], bf16, name="rnd_own")
        for half in range(2):
            pn = ps_n.tile([4, PANEL], f32, tag="pn")
            for c in range(CHD):
                nc.tensor.matmul(
                    pn[:],
                    w_sq[:, c, :],
                    xsq_own[:, c, 4 * half : 4 * half + 4, :],
                    start=(c == 0),
                    stop=(c == CHD - 1),
                )
            s = smal.tile([4, PANEL], f32, tag="s")
            nc.vector.reciprocal_approx_fast(s[:], pn[:])
            rn = smal.tile([4, PANEL], bf16, tag="rn")
            nc.scalar.activation(rn[:], s[:], Sqrt, scale=S2 * S2)
            nc.sync.dma_start(
                rnd_own[:, half * PANEL : (half + 1) * PANEL], rn[:]
            )
        bc_own = consts.tile([P, H, NBLK * P], bf16)
        nc.sync.dma_start(
            bc_own[:],
            bass.AP(
                rnd_own.tensor,
                rnd_own.offset,
                [[0, P], [NBLK * P, H], [1, NBLK * P]],
            ),
        )

        for j in range(2, PIPE):
            square(ORDER[j])
            norm_mm(ORDER[j])
            rnorm_bounce(ORDER[j])

        # tmp_all[kc] = xTown * asq[kc] for all 8 blocks: depends only on
        # inputs, so it runs in the prologue off every critical path.
        tmp_all = consts.tile([P, KCH, NBLK, P], bf16)
        for kc in range(KCH):
            nc.vector.tensor_scalar_mul(
                tmp_all[:, kc, :, :],
                xTown[:, kc % CHD, :, :],
                asq[:, kc : kc + 1],
            )

        rhs_build(ORDER[0])

        # lhsT build in block PAIRS (r, r+1): block r is first consumed at
        # panel 2r, so pair b covers panels 4b..; built just in time.
        def lhsT_build(pair, h, c, eng):
            blks = slice(2 * pair, 2 * pair + 2)
            kc = h * CHD + c
            eng.tensor_tensor(
                lhsT[:, blks, h, c, :],
                tmp_all[:, kc, blks, :],
                bc_own[:, h, 2 * pair * P : (2 * pair + 2) * P].rearrange(
                    "q (i n) -> q i n", n=P
                ),
                mybir.AluOpType.mult,
            )

        # pair 0 on GpSimd (idle at startup); pairs 1-3 on DVE right after
        # rhs(15): main(15) consumes blocks in order r=0..7, so later pairs
        # may lag slightly without stalling the PE for long.
        for h in range(H):
            for c in range(CHD):
                lhsT_build(0, h, c, nc.gpsimd)
        for pair in (3,):
            for h in range(H):
                for c in range(CHD):
                    lhsT_build(pair, h, c, nc.gpsimd)
        for pair in (1, 2):
            for h in range(H):
                for c in range(CHD):
                    lhsT_build(pair, h, c, nc.vector)
        square(ORDER[PIPE])

        # ---- main loop over 16 column panels, heaviest (p=15) first --------
        copy_cnt = 0
        for j in range(NPANELS):
            p = ORDER[j]
            if j + PIPE + 1 < NPANELS:
                load_panel(ORDER[j + PIPE + 1])
            if j + 1 < NPANELS and ORDER[j + 1] not in POOL_RHS:
                rhs_build(ORDER[j + 1])
            if j + 2 < NPANELS and ORDER[j + 2] in POOL_RHS:
                rhs_build(ORDER[j + 2])
            # norm matmul BEFORE this panel's main matmuls: its square was
            # issued an iteration earlier, so the PE never waits here and
            # the dependent Act sqrt clears before this panel's copies.
            if j + PIPE < NPANELS:
                norm_mm(ORDER[j + PIPE])

            k_p = p // 2 + 1
            rhs = rhss.pop(p)
            ot = outp.tile([P, NBLK, PANEL], f16, tag="ot")
            for r in range(k_p):
                acc = ps_o.tile([P, PANEL], f32, tag="acc")
                for kk in range(NPAIR):
                    nc.tensor.matmul(
                        acc[:],
                        lhsT[:, r, kk, :, :],
                        rhs[:, kk, :, :],
                        start=(kk == 0),
                        stop=(kk == NPAIR - 1),
                        perf_mode=DR,
                    )
                on_dve = (copy_cnt % 2 == 0) if p <= 1 else False
                if on_dve:
                    nc.vector.tensor_scalar_mul(ot[:, r, :], acc[:], OSCALE)
                else:
                    nc.scalar.mul(ot[:, r, :], acc[:], OSCALE)
                copy_cnt += 1
                if p <= 1:
                    nc.sync.dma_start(
                        out[r * P : (r + 1) * P, p * PANEL : (p + 1) * PANEL],
                        ot[:, r, :],
                    )
            if p > 1:
                nc.sync.dma_start(
                    out[0 : k_p * P, p * PANEL : (p + 1) * PANEL].rearrange(
                        "(r q) n -> q r n", q=P
                    ),
                    ot[:, :k_p, :],
                )

            # rest of the prepass pipeline
            if j + PIPE < NPANELS:
                rnorm_bounce(ORDER[j + PIPE])
            if j + PIPE + 1 < NPANELS:
                square(ORDER[j + PIPE + 1])

    nc.compile()
    return nc


def _get_compiled():
    if "nc" not in _COMPILED:
        _COMPILED["nc"] = _build_bass()
    return _COMPILED["nc"]


def host_side_inputs(x, attn):
    """Per-core input maps. Host work is layout/dtype marshaling of x plus
    tiny functions of attn_vectors; all real compute (norms, scaling, the
    N^2 matmul) runs on device."""
    import ml_dtypes

    bf16 = ml_dtypes.bfloat16
    xT = np.ascontiguousarray(x.T).astype(bf16)  # [256, 8192]
    w_sq = np.zeros((P, CHD, H), dtype=np.float32)
    asq = np.zeros((P, KCH), dtype=np.float32)
    for c in range(CHD):
        for h in range(H):
            w_sq[:, c, h] = attn[h, c * P : (c + 1) * P] ** 2
            asq[:, h * CHD + c] = attn[h, c * P : (c + 1) * P] ** 2
    w_sq = w_sq.astype(bf16)
    maps = []
    for c in range(NCORES):
        cols = np.concatenate(
            [
                np.arange((8 * i + c) * P, (8 * i + c + 1) * P)
                for i in range(NBLK)
            ]
        )
        maps.append(
            {
                "xT": xT,
                "xT_own": np.ascontiguousarray(xT[:, cols]),
                "w_sq": w_sq,
                "asq": asq,
            }
        )
    return maps


def assemble_output(results):
    """Scatter per-core row blocks into the full matrix and mirror the
    computed upper triangle."""
    full = np.empty((N, N), dtype=np.float16)
    for c in range(NCORES):
        o = results[c]["out"]
        for i in range(NBLK):
            full[(8 * i + c) * P : (8 * i + c + 1) * P, :] = o[
                i * P : (i + 1) * P, :
            ]
    f = full.astype(np.float32)
    return np.triu(f) + np.triu(f, 1).T


def kernel(**inputs) -> np.ndarray:
    from concourse import bass_utils

    x = np.ascontiguousarray(np.asarray(inputs["x"], dtype=np.float32))
    attn = np.ascontiguousarray(
        np.asarray(inputs["attn_vectors"], dtype=np.float32)
    )
    nc = _get_compiled()
    res = bass_utils.run_bass_kernel_spmd(
        nc, host_side_inputs(x, attn), core_ids=list(range(NCORES))
    )
    return assemble_output(res.results)


# revision 25
# speedup vs baseline: 1.2405x; 1.0099x over previous
"""Self-contained Trainium2 Bass kernel for the "Attentive" GNN message-passing
problem:

    x: [8192, 256] f32, attn_vectors: [4, 256] f32
    e_h = l2_normalize(attn_vectors[h] * x, axis=-1)        # [H, N, D]
    Y   = concat_h(e_h)                                     # [N, H*D]
    out = (Y @ Y.T) / H                                     # [N, N]

Strategy (8 NeuronCores, SPMD, no collectives):
  - out is symmetric: only the upper triangle is computed on-device, the host
    mirrors it.  Row blocks are dealt block-cyclically: core c owns global
    128-row blocks {8i + c}; its i-th block computes column panels [2i, 15].
    Every core runs the IDENTICAL program (k_p = p//2 + 1 blocks per panel,
    72 of the 128 baseline blocks per core).
  - Main matmul runs in fp8 e4m3 with DoubleRow perf mode (two 128-deep
    k-chunks per pass, 2x bf16 throughput, measured 216 ns per
    [K=256]x[128x512]); PSUM accumulates f32; out is written f16 and upcast
    on the host (tolerance gate is 2e-2; this pipeline measures ~1.3e-2).
  - x arrives pre-transposed/bf16 from the host (xT [256, 8192]); per-panel
    loads are plain strided DMAs, so there are no PE transposes or casts.
  - Row norms: xsq = Act Square(xT); pn[h, rows] via [4,512]-output bf16
    matmuls (w_sq stationary); rn = sqrt(recip_approx_fast(pn))*S2 =
    S2/||a_h*x_row|| (recip first: the custom DVE op is f32-only, the Act
    sqrt then casts to bf16 for free); rn bounces through DRAM and returns
    as a partition-broadcast DMA (step-0 partition APs are legal on DRAM).
  - Panels are processed in REVERSE order (15..0): the k=8 panels run while
    the rhs lookahead fills, and the rhs-starved k=1 panels land in the
    tail where the PE has nothing else to do anyway.
  - fp8 scales: lhsT = x*a^2*r*(S1/4), rhs = x*r*S2, with S1=64, S2=16 and
    S1/(4*S2) == 1 folded into asq; the PSUM->SBUF copy applies 1/(S1*S2).
  - Elementwise work is spread: rhs panel builds on DVE + GpSimd, squares +
    sqrt + most output copies on Scalar (sqrt/copy/square share one act
    table), recip + some copies on DVE.
"""

from contextlib import ExitStack

import numpy as np

N, D, H = 8192, 256, 4
NCORES = 8
P = 128
PANEL = 512
NPANELS = N // PANEL  # 16
NBLK = 8  # local row blocks per core
CHD = D // P  # 2
KCH = H * CHD  # 8 contraction chunks of 128
NPAIR = KCH // 2  # 4 DoubleRow pairs
S1 = 64.0
S2 = 16.0
OSCALE = 1.0 / (S1 * S2)
PIPE = 6  # panels of prepass lookahead
# panels whose rhs build runs on GpSimd instead of DVE
POOL_RHS = {13, 10, 7}

_COMPILED = {}


def _build_bass():
    import concourse.bass as bass
    import concourse.tile as tile
    from concourse import bacc, mybir

    f32 = mybir.dt.float32
    bf16 = mybir.dt.bfloat16
    f16 = mybir.dt.float16
    fp8 = mybir.dt.float8e4
    DR = mybir.MatmulPerfMode.DoubleRow
    Sqrt = mybir.ActivationFunctionType.Sqrt
    Square = mybir.ActivationFunctionType.Square

    nc = bacc.Bacc(
        "TRN2",
        target_bir_lowering=False,
        debug=False,
        enable_asserts=False,
        num_devices=NCORES,
    )
    xT_t = nc.dram_tensor("xT", [D, N], bf16, kind="ExternalInput")
    xo_t = nc.dram_tensor("xT_own", [D, NBLK * P], bf16, kind="ExternalInput")
    ws_t = nc.dram_tensor("w_sq", [P, CHD, H], bf16, kind="ExternalInput")
    aq_t = nc.dram_tensor("asq", [P, KCH], f32, kind="ExternalInput")
    out_t = nc.dram_tensor("out", [NBLK * P, N], f16, kind="ExternalOutput")

    xTa, out = xT_t.ap(), out_t.ap()

    with tile.TileContext(nc) as tc, ExitStack() as ctx, nc.allow_low_precision(
        "fp8 kernel by design"
    ):
        consts = ctx.enter_context(tc.tile_pool(name="consts", bufs=1))
        xtp = ctx.enter_context(tc.tile_pool(name="xtp", bufs=10))
        sqp = ctx.enter_context(tc.tile_pool(name="sqp", bufs=4))
        smal = ctx.enter_context(tc.tile_pool(name="smal", bufs=3))
        bcp = ctx.enter_context(tc.tile_pool(name="bcp", bufs=8))
        rhsp = ctx.enter_context(tc.tile_pool(name="rhsp", bufs=7))
        outp = ctx.enter_context(tc.tile_pool(name="outp", bufs=3))
        tmpp = ctx.enter_context(tc.tile_pool(name="tmpp", bufs=2))
        dram = ctx.enter_context(tc.tile_pool(name="dram", bufs=1, space="DRAM"))
        ps_n = ctx.enter_context(tc.tile_pool(name="ps_n", bufs=2, space="PSUM"))
        ps_o = ctx.enter_context(tc.tile_pool(name="ps_o", bufs=6, space="PSUM"))

        # lhsT: [128, block r, pair kk, t, m] fp8  (pair stride = 128, the
        # layout the dual-fp8 ldweights path was validated with)
        lhsT = consts.tile([P, NBLK, NPAIR, 2, P], fp8)
        w_sq = consts.tile([P, CHD, H], bf16)
        asq = consts.tile([P, KCH], f32)
        xTown = consts.tile([P, CHD, NBLK, P], bf16)

        xts = {}
        bcs = {}
        rhss = {}
        sqs = {}
        pns = {}

        def load_panel(p):
            t = xtp.tile([P, CHD, PANEL], bf16, tag="xt")
            nc.sync.dma_start(
                t[:],
                xTa[:, p * PANEL : (p + 1) * PANEL].rearrange(
                    "(c q) n -> q c n", q=P
                ),
            )
            xts[p] = t

        def square(p):
            sq = sqp.tile([P, CHD, PANEL], bf16, tag="sq")
            nc.scalar.activation(sq[:], xts[p][:], Square)
            sqs[p] = sq

        def norm_mm(p):
            pn = ps_n.tile([4, PANEL], f32, tag="pn")
            for c in range(CHD):
                nc.tensor.matmul(
                    pn[:],
                    w_sq[:, c, :],
                    sqs[p][:, c, :],
                    start=(c == 0),
                    stop=(c == CHD - 1),
                )
            pns[p] = pn

        def rnorm_bounce(p):
            # rn = S2/sqrt(pn): DVE reciprocal first (needs f32 in/out),
            # then Act sqrt with scale=S2^2 casting to bf16 for free.
            s = smal.tile([4, PANEL], f32, tag="s")
            nc.vector.reciprocal_approx_fast(s[:], pns[p][:])
            rn = smal.tile([4, PANEL], bf16, tag="rn")
            nc.scalar.activation(rn[:], s[:], Sqrt, scale=S2 * S2)
            rnd = dram.tile([4, PANEL], bf16, name=f"rnd{p}")
            nc.sync.dma_start(rnd[:], rn[:])
            bc = bcp.tile([P, H, PANEL], bf16, tag="bc")
            src = bass.AP(rnd.tensor, rnd.offset, [[0, P], [PANEL, H], [1, PANEL]])
            nc.sync.dma_start(bc[:], src)
            bcs[p] = bc

        def rhs_build(p):
            # rhs[q, h, c, n] = xT[q, c, n] * bc[q, h, n]
            rhs = rhsp.tile([P, NPAIR, 2, PANEL], fp8, tag="rhs")
            xt = xts[p]
            bc = bcs[p]
            in0 = bass.AP(
                xt.tensor,
                xt.offset,
                [list(xt.ap[0]), [0, H], [PANEL, CHD], [1, PANEL]],
            )
            in1 = bass.AP(
                bc.tensor,
                bc.offset,
                [list(bc.ap[0]), [PANEL, H], [0, CHD], [1, PANEL]],
            )
            eng = nc.gpsimd if p in POOL_RHS else nc.vector
            eng.tensor_tensor(rhs[:], in0, in1, mybir.AluOpType.mult)
            rhss[p] = rhs

        # ---- prologue (panels processed in REVERSE order 15..0) -----------
        # DMA trigger order: the panel-15/14 loads and xTown lead (they gate
        # the startup chains), small consts after.
        ORDER = list(range(NPANELS - 1, -1, -1))
        load_panel(ORDER[0])
        load_panel(ORDER[1])
        nc.sync.dma_start(
            xTown[:],
            xo_t.ap()[:].rearrange("(c q) (i n) -> q c i n", q=P, n=P),
        )
        nc.sync.dma_start(w_sq[:], ws_t.ap()[:])
        nc.sync.dma_start(asq[:], aq_t.ap()[:])
        for j in range(2, PIPE + 1):
            load_panel(ORDER[j])

        # Dummy tiny sqrt: pins the act table that holds sqrt+square+copy
        # before the panel-0 chain, so no reload lands on the critical path.
        dummy = smal.tile([4, 8], f32, tag="dummy")
        nc.scalar.activation(dummy[:], asq[:4, :], Sqrt)

        # Panel 15/14 prepass chains first: rhs(15) is the main-loop gate.
        for j in range(2):
            square(ORDER[j])
            norm_mm(ORDER[j])
            rnorm_bounce(ORDER[j])

        # Own-row norm pass (gates only the lhsT build).
        xsq_own = consts.tile([P, CHD, NBLK, P], bf16)
        nc.scalar.activation(xsq_own[:], xTown[:], Square)
        rnd_own = dram.tile([4, NBLK * P], bf16, name="rnd_own")
        bc_own = consts.tile([P, H, NBLK * P], bf16)
        for half in range(2):
            pn = ps_n.tile([4, PANEL], f32, tag="pn")
            for c in range(CHD):
                nc.tensor.matmul(
                    pn[:],
                    w_sq[:, c, :],
                    xsq_own[:, c, 4 * half : 4 * half + 4, :],
                    start=(c == 0),
                    stop=(c == CHD - 1),
                )
            s = smal.tile([4, PANEL], f32, tag="s")
            nc.vector.reciprocal_approx_fast(s[:], pn[:])
            rn = smal.tile([4, PANEL], bf16, tag="rn")
            nc.scalar.activation(rn[:], s[:], Sqrt, scale=S2 * S2)
            nc.sync.dma_start(
                rnd_own[:, half * PANEL : (half + 1) * PANEL], rn[:]
            )
            nc.sync.dma_start(
                bc_own[:, :, half * PANEL : (half + 1) * PANEL],
                bass.AP(
                    rnd_own.tensor,
                    rnd_own.offset + half * PANEL,
                    [[0, P], [NBLK * P, H], [1, PANEL]],
                ),
            )

        for j in range(2, PIPE):
            square(ORDER[j])
            norm_mm(ORDER[j])
            rnorm_bounce(ORDER[j])

        # tmp_all[kc] = xTown * asq[kc] for all 8 blocks: depends only on
        # inputs, so it runs in the prologue off every critical path.
        tmp_all = consts.tile([P, KCH, NBLK, P], bf16)
        for kc in range(KCH):
            nc.vector.tensor_scalar_mul(
                tmp_all[:, kc, :, :],
                xTown[:, kc % CHD, :, :],
                asq[:, kc : kc + 1],
            )

        rhs_build(ORDER[0])

        # lhsT build in block PAIRS (r, r+1): block r is first consumed at
        # panel 2r, so pair b covers panels 4b..; built just in time.
        def lhsT_build(pair, h, c, eng):
            blks = slice(2 * pair, 2 * pair + 2)
            kc = h * CHD + c
            eng.tensor_tensor(
                lhsT[:, blks, h, c, :],
                tmp_all[:, kc, blks, :],
                bc_own[:, h, 2 * pair * P : (2 * pair + 2) * P].rearrange(
                    "q (i n) -> q i n", n=P
                ),
                mybir.AluOpType.mult,
            )

        # pair 0 on GpSimd (idle at startup); pairs 1-3 on DVE right after
        # rhs(15): main(15) consumes blocks in order r=0..7, so later pairs
        # may lag slightly without stalling the PE for long.
        for h in range(H):
            for c in range(CHD):
                lhsT_build(0, h, c, nc.gpsimd)
        for pair in (3,):
            for h in range(H):
                for c in range(CHD):
                    lhsT_build(pair, h, c, nc.gpsimd)
        for pair in (1, 2):
            for h in range(H):
                for c in range(CHD):
                    lhsT_build(pair, h, c, nc.vector)
        square(ORDER[PIPE])

        # ---- main loop over 16 column panels, heaviest (p=15) first --------
        copy_cnt = 0
        for j in range(NPANELS):
            p = ORDER[j]
            if j + PIPE + 1 < NPANELS:
                load_panel(ORDER[j + PIPE + 1])
            if j + 1 < NPANELS and ORDER[j + 1] not in POOL_RHS:
                rhs_build(ORDER[j + 1])
            if j + 2 < NPANELS and ORDER[j + 2] in POOL_RHS:
                rhs_build(ORDER[j + 2])
            # norm matmul BEFORE this panel's main matmuls: its square was
            # issued an iteration earlier, so the PE never waits here and
            # the dependent Act sqrt clears before this panel's copies.
            if j + PIPE < NPANELS:
                norm_mm(ORDER[j + PIPE])

            k_p = p // 2 + 1
            rhs = rhss.pop(p)
            ot = outp.tile([P, NBLK, PANEL], f16, tag="ot")
            for r in range(k_p):
                acc = ps_o.tile([P, PANEL], f32, tag="acc")
                for kk in range(NPAIR):
                    nc.tensor.matmul(
                        acc[:],
                        lhsT[:, r, kk, :, :],
                        rhs[:, kk, :, :],
                        start=(kk == 0),
                        stop=(kk == NPAIR - 1),
                        perf_mode=DR,
                    )
                on_dve = (copy_cnt % 2 == 0) if p <= 1 else False
                if on_dve:
                    nc.vector.tensor_scalar_mul(ot[:, r, :], acc[:], OSCALE)
                else:
                    nc.scalar.mul(ot[:, r, :], acc[:], OSCALE)
                copy_cnt += 1
                if p <= 1:
                    nc.sync.dma_start(
                        out[r * P : (r + 1) * P, p * PANEL : (p + 1) * PANEL],
                        ot[:, r, :],
                    )
            if p > 1:
                nc.sync.dma_start(
                    out[0 : k_p * P, p * PANEL : (p + 1) * PANEL].rearrange(
                        "(r q) n -> q r n", q=P
                    ),
                    ot[:, :k_p, :],
                )

            # rest of the prepass pipeline
            if j + PIPE < NPANELS:
                rnorm_bounce(ORDER[j + PIPE])
            if j + PIPE + 1 < NPANELS:
                square(ORDER[j + PIPE + 1])

    nc.compile()
    return nc


def _get_compiled():
    if "nc" not in _COMPILED:
        _COMPILED["nc"] = _build_bass()
    return _COMPILED["nc"]


def host_side_inputs(x, attn):
    """Per-core input maps. Host work is layout/dtype marshaling of x plus
    tiny functions of attn_vectors; all real compute (norms, scaling, the
    N^2 matmul) runs on device."""
    import ml_dtypes

    bf16 = ml_dtypes.bfloat16
    xT = np.ascontiguousarray(x.T).astype(bf16)  # [256, 8192]
    w_sq = np.zeros((P, CHD, H), dtype=np.float32)
    asq = np.zeros((P, KCH), dtype=np.float32)
    for c in range(CHD):
        for h in range(H):
            w_sq[:, c, h] = attn[h, c * P : (c + 1) * P] ** 2
            asq[:, h * CHD + c] = attn[h, c * P : (c + 1) * P] ** 2
    w_sq = w_sq.astype(bf16)
    maps = []
    for c in range(NCORES):
        cols = np.concatenate(
            [
                np.arange((8 * i + c) * P, (8 * i + c + 1) * P)
                for i in range(NBLK)
            ]
        )
        maps.append(
            {
                "xT": xT,
                "xT_own": np.ascontiguousarray(xT[:, cols]),
                "w_sq": w_sq,
                "asq": asq,
            }
        )
    return maps


def assemble_output(results):
    """Scatter per-core row blocks into the full matrix and mirror the
    computed upper triangle."""
    full = np.empty((N, N), dtype=np.float16)
    for c in range(NCORES):
        o = results[c]["out"]
        for i in range(NBLK):
            full[(8 * i + c) * P : (8 * i + c + 1) * P, :] = o[
                i * P : (i + 1) * P, :
            ]
    f = full.astype(np.float32)
    return np.triu(f) + np.triu(f, 1).T


def kernel(**inputs) -> np.ndarray:
    from concourse import bass_utils

    x = np.ascontiguousarray(np.asarray(inputs["x"], dtype=np.float32))
    attn = np.ascontiguousarray(
        np.asarray(inputs["attn_vectors"], dtype=np.float32)
    )
    nc = _get_compiled()
    res = bass_utils.run_bass_kernel_spmd(
        nc, host_side_inputs(x, attn), core_ids=list(range(NCORES))
    )
    return assemble_output(res.results)
